# revision 1
# baseline (speedup 1.0000x reference)
"""ChamferLoss (cosine) Trainium2 kernel.

Math: for clouds a, b in [B, N, 3],
  per direction: for each point x in a, smax = max_m cos(x, b_m);
  d = (1 - min(smax, 1))^2; loss = sum over points/directions/batches / (N*B).
Since (1 - min(s,1))^2 is monotone non-increasing in s, min_m d == d(max_m sim).

Device strategy (8 cores, batch-parallel, 2 batches/core):
  - host passes batch-transposed [3, 4096] arrays (layout prep only)
  - on-chip: all 4 clouds live in one [8, 4*3*512] "block-row" tile
    (partition j = 512-point block, free = (matrix, component, offset));
    norms/rsqrt/scaling are pure free-dim ops on partitions 0-7;
    one DRAM bounce (single store + single load) rearranges everything
    into one [3, 4*4096] PE-operand tile at partitions 0-2
  - main loop: K=3 fp32 matmuls (exact; fp32r/tf32 rounding is NOT precise
    enough for (1-s)^2 when 1-s ~ 5e-4) producing 128x512 similarity tiles
    in PSUM; DVE reduce_max over 4-bank [128, 2048] groups; both directions
    get their own matmul pass
  - epilogue: clamp, (1-s)^2, partial sums; host sums 8x[128] partials.

Every DMA lands on its own HW-DGE lane (5 total), which keeps every
instruction's sync-wait count within walrus' per-instruction limits and
avoids Tile's lane-reuse tick bookkeeping; _split_fat_waits() repairs the
remaining over-limit instructions (matmul slot-reuse waits, kernel-tail
drain) after tracing.
"""

import numpy as np

B = 16
N = 4096
C = 3
CORES = 8
B_PER_CORE = B // CORES
NBLK = 8          # 512-point database blocks per cloud
BLK = 512
NI = N // 128     # 128-point query chunks per cloud
NMAT = 2 * B_PER_CORE          # clouds resident per core
MFREE = C * BLK                # 1536, per-cloud chunk in xt/xh tiles

_CACHE = {}


def _set_waits(inst, waits):
    import bass_rust
    si = inst.sync_info
    upd = list(si.on_update) if si is not None else []
    inst.sync_info = bass_rust.SyncInfo(on_wait=waits, on_update=upd)


def _split_fat_waits(nc):
    """walrus (CoreV3) caps sync waits per instruction (1 for Matmult, 2 for
    everything else), but Tile can emit more.  Two repairs:

    1. Matmults with 2 waits: keep the PE-self wait, move the cross-engine
       wait onto the nearest PRECEDING PE instruction with spare budget.
       PE executes in order, so the condition still holds before the
       matmult issues; the moved wait's producer reads a PSUM group two
       generations older than the host, so no cycle is possible.

    2. Kernel-tail drain with one wait per outstanding semaphore:
       redistribute the excess onto instructions the semaphore reset
       transitively waits on (barrier Drains + Pool-stream instructions
       preceding the reset)."""
    import concourse.mybir as mybir

    ENG_PFX = {
        "PE": "PE_", "DVE": "DVE_", "Activation": "Activation_",
        "Pool": "Pool_", "SP": "SP_",
    }

    def budget(inst):
        return 1 if inst.opcode in ("Matmult", "Drain") else 2

    for blk in nc.main_func.blocks:
        insts = list(blk.instructions)

        # ---- repair 1: over-budget engine instructions (main body) ----
        for idx, inst in enumerate(insts):
            si = inst.sync_info
            if si is None:
                continue
            waits = list(si.on_wait)
            b = budget(inst)
            if len(waits) <= b or inst.opcode == "Drain":
                continue
            # keep the self-engine wait (moving those backward can deadlock),
            # move cross-engine waits onto preceding same-engine instructions
            eng = inst.engine
            pfx = ENG_PFX.get(str(eng).split(".")[-1], "\0")
            keep = [w for w in waits if w.ant_name.startswith(pfx)][:b]
            if len(keep) < b:
                keep += [w for w in waits if w not in keep][:b - len(keep)]
            excess = [w for w in waits if w not in keep]
            _set_waits(inst, keep)
            back = idx - 1
            hops = 0
            while excess and back >= 0 and hops < 16:
                h = insts[back]
                back -= 1
                if h.engine != eng or not h.is_executable():
                    continue
                hops += 1
                hsi = h.sync_info
                hw = list(hsi.on_wait) if hsi is not None else []
                hb = budget(h)
                while len(hw) < hb and excess:
                    hw.append(excess.pop(0))
                _set_waits(h, hw)
            assert not excess, f"no host for waits of {inst.name}"

        # ---- repair 2: the kernel-tail fat drain (Drain budget is 1) ----
        # The barrier butterfly that follows flushes every engine pipeline,
        # so engine-sem waits on the tail drain are redundant.  DMA-lane sems
        # are transitively quiesced when a covered instruction waited for
        # their final value (fixpoint below); only genuinely unconsumed
        # lanes (e.g. the output DMA) need explicit tail waits.
        fat_idx = None
        for idx, inst in enumerate(insts):
            si = inst.sync_info
            if si is not None and len(si.on_wait) > 1 and inst.opcode == "Drain":
                fat_idx = idx
                break
        if fat_idx is None:
            continue
        fat = insts[fat_idx]
        waits = list(fat.sync_info.on_wait)
        eng_prefixes = ("Activation", "PE_", "DVE_", "Pool_", "SP_")
        lane_waits = [w for w in waits
                      if not w.ant_name.startswith(eng_prefixes)]
        # coverage fixpoint over DMA-lane sems (program-wide scan)
        all_waits = []           # (host_inst, sem_name, value)
        for blk2 in nc.main_func.blocks:
            for inst in blk2.instructions:
                if inst.name == fat.name:
                    continue
                isi = inst.sync_info
                if isi is None:
                    continue
                for w in isi.on_wait:
                    all_waits.append((inst, w.ant_name, w.wait_value))
        covered = set()
        changed = True
        while changed:
            changed = False
            for w in lane_waits:
                if w.ant_name in covered:
                    continue
                for host, sem, val in all_waits:
                    if sem != w.ant_name or val < w.wait_value:
                        continue
                    # host counts if it is an engine instruction, or a DMA
                    # whose own lane is covered
                    hsi = host.sync_info
                    hticks = [getattr(u, "ant_name", "") for u in
                              (hsi.on_update if hsi else [])]
                    hlanes = [t for t in hticks
                              if t and not t.startswith(eng_prefixes)]
                    if all(t in covered for t in hlanes):
                        covered.add(w.ant_name)
                        changed = True
                        break
        excess = [w for w in lane_waits if w.ant_name not in covered]
        _set_waits(fat, excess[:1])
        excess = excess[1:]
        for inst in insts[fat_idx + 1:]:
            if not excess:
                break
            if getattr(inst, "is_reset_sema", False):
                break
            if inst.engine != mybir.EngineType.Pool:
                continue
            isi = inst.sync_info
            cur_w = list(isi.on_wait) if isi is not None else []
            if len(cur_w) >= 1:
                continue
            cur_w.append(excess.pop(0))
            _set_waits(inst, cur_w)
        assert not excess, f"could not place {len(excess)} tail waits"


def _build():
    import concourse.bass as bass
    import concourse.mybir as mybir
    import concourse.tile as tile
    from contextlib import ExitStack

    f32 = mybir.dt.float32
    f32r = mybir.dt.float32r
    AX = mybir.AxisListType
    AF = mybir.ActivationFunctionType
    OP = mybir.AluOpType

    nc = bass.Bass("TRN2", target_bir_lowering=False, debug=False)
    m1t = nc.dram_tensor("m1t", [B_PER_CORE, C, N], f32, kind="ExternalInput")
    m2t = nc.dram_tensor("m2t", [B_PER_CORE, C, N], f32, kind="ExternalInput")
    out = nc.dram_tensor("out", [128, 1], f32, kind="ExternalOutput")

    with tile.TileContext(nc) as tc, ExitStack() as ctx:
        sb = ctx.enter_context(tc.tile_pool(name="sb", bufs=1))
        ps_pool = ctx.enter_context(tc.tile_pool(name="ps", bufs=2, space="PSUM"))
        dr = ctx.enter_context(tc.tile_pool(name="dr", bufs=1, space="DRAM"))

        # ---- prologue ----
        # xt_all[j, mi*2*MFREE + bb*MFREE + c*BLK + f] = X_{mi,bb}[c, 512j+f]
        xt_all = sb.tile([NBLK, NMAT * MFREE + 8], f32)
        for mi, src in enumerate((m1t, m2t)):
            dst = xt_all[:, mi * B_PER_CORE * MFREE:
                         (mi + 1) * B_PER_CORE * MFREE]
            nc.sync.dma_start(
                dst.rearrange("j (bb c f) -> j bb c f", c=C, f=BLK),
                src.ap().rearrange("bb c (j f) -> j bb c f", f=BLK))

        def moff(bb, mi):
            return (mi * B_PER_CORE + bb) * MFREE

        # xh_all: normalized, free layout (m, c, f) with m = bb*2 + mi
        xh_all = sb.tile([NBLK, NMAT * MFREE + 8], f32)
        for bb in range(B_PER_CORE):
            for mi in range(2):
                off = moff(bb, mi)
                sfx = f"{bb}_{mi}"
                na2 = sb.tile([NBLK, BLK], f32, name=f"na2_{sfx}")
                sq1 = sb.tile([NBLK, BLK], f32, name=f"sq1_{sfx}")
                sq2 = sb.tile([NBLK, BLK], f32, name=f"sq2_{sfx}")
                xs = [xt_all[:, off + c * BLK: off + (c + 1) * BLK]
                      for c in range(C)]
                nc.vector.tensor_tensor(na2[:], xs[0], xs[0], OP.mult)
                nc.vector.tensor_tensor(sq1[:], xs[1], xs[1], OP.mult)
                nc.vector.tensor_tensor(sq2[:], xs[2], xs[2], OP.mult)
                nc.vector.tensor_tensor(na2[:], na2[:], sq1[:], OP.add)
                nc.vector.tensor_tensor(na2[:], na2[:], sq2[:], OP.add)

                # rn = 1/sqrt(na2): r = 1/na2 (iterative divide, accurate),
                # y0 = ACT sqrt(r), one Newton step y1 = 0.5*(y0 + r/y0)
                r = sb.tile([NBLK, BLK], f32, name=f"r_{sfx}")
                nc.vector.reciprocal(r[:], na2[:])
                y0 = sb.tile([NBLK, BLK], f32, name=f"y0_{sfx}")
                nc.scalar.sqrt(y0[:], r[:])
                iy = sb.tile([NBLK, BLK], f32, name=f"iy_{sfx}")
                nc.vector.reciprocal(iy[:], y0[:])
                nc.vector.tensor_tensor(iy[:], iy[:], r[:], OP.mult)
                nc.vector.tensor_tensor(iy[:], iy[:], y0[:], OP.add)
                rn = sb.tile([NBLK, BLK], f32, name=f"rn_{sfx}")
                nc.vector.tensor_scalar_mul(rn[:], iy[:], 0.5)

                for c in range(C):
                    nc.vector.tensor_tensor(
                        xh_all[:, off + c * BLK: off + (c + 1) * BLK],
                        xs[c], rn[:], OP.mult)

        # single-pass fp32 matmuls (4 cyc/row on PE, but exact):
        # one DRAM bounce rearranges the normalized clouds into one
        # [3, 4*4096] PE-operand tile at partitions 0-2
        scr = dr.tile([NMAT, NBLK, C, BLK], f32)
        nc.sync.dma_start(
            scr[:].rearrange("m j c f -> j m c f"),
            xh_all[:, 0:NMAT * MFREE]
            .rearrange("j (m c f) -> j m c f", c=C, f=BLK))
        x3_all = sb.tile([C, NMAT * N + 8], f32)
        nc.sync.dma_start(
            x3_all[:, 0:NMAT * N].rearrange("c (m j f) -> c m j f",
                                            j=NBLK, f=BLK),
            scr[:].rearrange("m j c f -> c m j f"))

        def x3(bb, mi):
            base = (mi * B_PER_CORE + bb) * N
            return x3_all[:, base:base + N]

        # ---- main: both directions per batch ----
        acc = sb.tile([128, 2 * B_PER_CORE], f32)
        k = 0
        for bb in range(B_PER_CORE):
            for (q, d) in ((0, 1), (1, 0)):
                qt = x3(bb, q)   # queries  [3, 4096]
                dt = x3(bb, d)   # database [3, 4096]
                rowparts = sb.tile([128, 2 * NI], f32, name=f"rp_{bb}_{q}")
                for i in range(NI):
                    lhsT = qt[:, i * 128:(i + 1) * 128]
                    for g in range(2):
                        psm = ps_pool.tile([128, 4 * BLK], f32, name="psm",
                                           tag="psm")
                        for jj in range(4):
                            blk = g * 4 + jj
                            rhs = dt[:, blk * BLK:(blk + 1) * BLK]
                            nc.tensor.matmul(
                                psm[:, jj * BLK:(jj + 1) * BLK],
                                lhsT=lhsT, rhs=rhs, start=True, stop=True)
                        nc.vector.reduce_max(
                            rowparts[:, 2 * i + g:2 * i + g + 1], psm[:],
                            axis=AX.X)
                # smax over the two groups, clamp, (1-s)^2, row-sum
                smax = sb.tile([128, NI], f32, name=f"sm_{bb}_{q}")
                nc.vector.reduce_max(
                    smax[:], rowparts[:].rearrange("p (i g) -> p i g", g=2),
                    axis=AX.X)
                nc.vector.tensor_scalar_min(smax[:], smax[:], 1.0)
                dd = sb.tile([128, NI], f32, name=f"dd_{bb}_{q}")
                nc.scalar.activation(dd[:], smax[:], AF.Square,
                                     bias=1.0, scale=-1.0)
                nc.vector.reduce_sum(acc[:, k:k + 1], dd[:], axis=AX.X)
                k += 1

        accf = sb.tile([128, 2], f32)
        nc.vector.reduce_sum(accf[:, 0:1], acc[:], axis=AX.X)
        nc.sync.dma_start(out.ap(), accf[:, 0:1])

    _split_fat_waits(nc)
    return nc


def kernel(matrix1: np.ndarray, matrix2: np.ndarray) -> np.ndarray:
    from concourse.bass_utils import run_bass_kernel_spmd

    if "nc" not in _CACHE:
        _CACHE["nc"] = _build()
    nc = _CACHE["nc"]

    m1t = np.ascontiguousarray(np.asarray(matrix1).transpose(0, 2, 1))  # [B,3,N]
    m2t = np.ascontiguousarray(np.asarray(matrix2).transpose(0, 2, 1))
    in_maps = []
    for c in range(CORES):
        sl = slice(B_PER_CORE * c, B_PER_CORE * (c + 1))
        in_maps.append({"m1t": m1t[sl], "m2t": m2t[sl]})
    res = run_bass_kernel_spmd(nc, in_maps, core_ids=list(range(CORES)))
    total = np.float64(0.0)
    for c in range(CORES):
        total += np.float64(res.results[c]["out"].sum(dtype=np.float64))
    return np.float32(total / (N * B))



# revision 2
# speedup vs baseline: 2.6297x; 2.6297x over previous
"""ChamferLoss (cosine) Trainium2 kernel.

Math: for clouds a, b in [B, N, 3],
  per direction: for each point x in a, smax = max_m cos(x, b_m);
  d = (1 - min(smax, 1))^2; loss = sum over points/directions/batches / (N*B).
Since (1 - min(s,1))^2 is monotone non-increasing in s, min_m d == d(max_m sim).

Device strategy (8 cores, batch-parallel, 2 batches/core):
  - host passes batch-transposed [3, 4096] arrays (layout prep only)
  - on-chip: all 4 clouds live in one [8, 4*3*512] "block-row" tile
    (partition j = 512-point block, free = (matrix, component, offset));
    norms/rsqrt/scaling are pure free-dim ops on partitions 0-7;
    one DRAM bounce (single store + single load) rearranges everything
    into one [3, 4*4096] PE-operand tile at partitions 0-2
  - main loop: K=3 fp32 matmuls (exact; fp32r/tf32 rounding is NOT precise
    enough for (1-s)^2 when 1-s ~ 5e-4) producing 128x512 similarity tiles
    in PSUM; DVE reduce_max over 4-bank [128, 2048] groups; both directions
    get their own matmul pass
  - epilogue: clamp, (1-s)^2, partial sums; host sums 8x[128] partials.

Every DMA lands on its own HW-DGE lane (5 total), which keeps every
instruction's sync-wait count within walrus' per-instruction limits and
avoids Tile's lane-reuse tick bookkeeping; _split_fat_waits() repairs the
remaining over-limit instructions (matmul slot-reuse waits, kernel-tail
drain) after tracing.
"""

import numpy as np

B = 16
N = 4096
C = 3
CORES = 8
B_PER_CORE = B // CORES
NBLK = 8          # 512-point database blocks per cloud
BLK = 512
NI = N // 128     # 128-point query chunks per cloud
NMAT = 2 * B_PER_CORE          # clouds resident per core
MFREE = C * BLK                # 1536, per-cloud chunk in xt/xh tiles

_CACHE = {}


def _set_waits(inst, waits):
    import bass_rust
    si = inst.sync_info
    upd = list(si.on_update) if si is not None else []
    inst.sync_info = bass_rust.SyncInfo(on_wait=waits, on_update=upd)


def _split_fat_waits(nc):
    """walrus (CoreV3) caps sync waits per instruction (1 for Matmult, 2 for
    everything else), but Tile can emit more.  Two repairs:

    1. Matmults with 2 waits: keep the PE-self wait, move the cross-engine
       wait onto the nearest PRECEDING PE instruction with spare budget.
       PE executes in order, so the condition still holds before the
       matmult issues; the moved wait's producer reads a PSUM group two
       generations older than the host, so no cycle is possible.

    2. Kernel-tail drain with one wait per outstanding semaphore:
       redistribute the excess onto instructions the semaphore reset
       transitively waits on (barrier Drains + Pool-stream instructions
       preceding the reset)."""
    import concourse.mybir as mybir

    ENG_PFX = {
        "PE": "PE_", "DVE": "DVE_", "Activation": "Activation_",
        "Pool": "Pool_", "SP": "SP_",
    }

    def budget(inst):
        return 1 if inst.opcode in ("Matmult", "Drain") else 2

    for blk in nc.main_func.blocks:
        insts = list(blk.instructions)

        # ---- repair 1: over-budget engine instructions (main body) ----
        for idx, inst in enumerate(insts):
            si = inst.sync_info
            if si is None:
                continue
            waits = list(si.on_wait)
            b = budget(inst)
            if len(waits) <= b or inst.opcode == "Drain":
                continue
            # keep the self-engine wait (moving those backward can deadlock),
            # move cross-engine waits onto preceding same-engine instructions
            eng = inst.engine
            pfx = ENG_PFX.get(str(eng).split(".")[-1], "\0")
            keep = [w for w in waits if w.ant_name.startswith(pfx)][:b]
            if len(keep) < b:
                keep += [w for w in waits if w not in keep][:b - len(keep)]
            excess = [w for w in waits if w not in keep]
            _set_waits(inst, keep)
            back = idx - 1
            hops = 0
            while excess and back >= 0 and hops < 16:
                h = insts[back]
                back -= 1
                if h.engine != eng or not h.is_executable():
                    continue
                hops += 1
                hsi = h.sync_info
                hw = list(hsi.on_wait) if hsi is not None else []
                hb = budget(h)
                while len(hw) < hb and excess:
                    hw.append(excess.pop(0))
                _set_waits(h, hw)
            assert not excess, f"no host for waits of {inst.name}"

        # ---- repair 2: the kernel-tail fat drain (Drain budget is 1) ----
        # The barrier butterfly that follows flushes every engine pipeline,
        # so engine-sem waits on the tail drain are redundant.  DMA-lane sems
        # are transitively quiesced when a covered instruction waited for
        # their final value (fixpoint below); only genuinely unconsumed
        # lanes (e.g. the output DMA) need explicit tail waits.
        fat_idx = None
        for idx, inst in enumerate(insts):
            si = inst.sync_info
            if si is not None and len(si.on_wait) > 1 and inst.opcode == "Drain":
                fat_idx = idx
                break
        if fat_idx is None:
            continue
        fat = insts[fat_idx]
        waits = list(fat.sync_info.on_wait)
        eng_prefixes = ("Activation", "PE_", "DVE_", "Pool_", "SP_")
        lane_waits = [w for w in waits
                      if not w.ant_name.startswith(eng_prefixes)]
        # coverage fixpoint over DMA-lane sems (program-wide scan)
        all_waits = []           # (host_inst, sem_name, value)
        for blk2 in nc.main_func.blocks:
            for inst in blk2.instructions:
                if inst.name == fat.name:
                    continue
                isi = inst.sync_info
                if isi is None:
                    continue
                for w in isi.on_wait:
                    all_waits.append((inst, w.ant_name, w.wait_value))
        covered = set()
        changed = True
        while changed:
            changed = False
            for w in lane_waits:
                if w.ant_name in covered:
                    continue
                for host, sem, val in all_waits:
                    if sem != w.ant_name or val < w.wait_value:
                        continue
                    # host counts if it is an engine instruction, or a DMA
                    # whose own lane is covered
                    hsi = host.sync_info
                    hticks = [getattr(u, "ant_name", "") for u in
                              (hsi.on_update if hsi else [])]
                    hlanes = [t for t in hticks
                              if t and not t.startswith(eng_prefixes)]
                    if all(t in covered for t in hlanes):
                        covered.add(w.ant_name)
                        changed = True
                        break
        excess = [w for w in lane_waits if w.ant_name not in covered]
        _set_waits(fat, excess[:1])
        excess = excess[1:]
        for inst in insts[fat_idx + 1:]:
            if not excess:
                break
            if getattr(inst, "is_reset_sema", False):
                break
            if inst.engine != mybir.EngineType.Pool:
                continue
            isi = inst.sync_info
            cur_w = list(isi.on_wait) if isi is not None else []
            if len(cur_w) >= 1:
                continue
            cur_w.append(excess.pop(0))
            _set_waits(inst, cur_w)
        assert not excess, f"could not place {len(excess)} tail waits"


def _build():
    import concourse.bass as bass
    import concourse.mybir as mybir
    import concourse.tile as tile
    from contextlib import ExitStack

    f32 = mybir.dt.float32
    f32r = mybir.dt.float32r
    AX = mybir.AxisListType
    AF = mybir.ActivationFunctionType
    OP = mybir.AluOpType

    nc = bass.Bass("TRN2", target_bir_lowering=False, debug=False)
    m1t = nc.dram_tensor("m1t", [B_PER_CORE, C, N], f32, kind="ExternalInput")
    m2t = nc.dram_tensor("m2t", [B_PER_CORE, C, N], f32, kind="ExternalInput")
    out = nc.dram_tensor("out", [128, 1], f32, kind="ExternalOutput")

    with tile.TileContext(nc) as tc, ExitStack() as ctx:
        sb = ctx.enter_context(tc.tile_pool(name="sb", bufs=1))
        ps_pool = ctx.enter_context(tc.tile_pool(name="ps", bufs=2, space="PSUM"))
        dr = ctx.enter_context(tc.tile_pool(name="dr", bufs=1, space="DRAM"))

        # ---- prologue ----
        # xt_all[j, mi*2*MFREE + bb*MFREE + c*BLK + f] = X_{mi,bb}[c, 512j+f]
        xt_all = sb.tile([NBLK, NMAT * MFREE + 8], f32)
        for mi, src in enumerate((m1t, m2t)):
            dst = xt_all[:, mi * B_PER_CORE * MFREE:
                         (mi + 1) * B_PER_CORE * MFREE]
            nc.sync.dma_start(
                dst.rearrange("j (bb c f) -> j bb c f", c=C, f=BLK),
                src.ap().rearrange("bb c (j f) -> j bb c f", f=BLK))

        def moff(bb, mi):
            return (mi * B_PER_CORE + bb) * MFREE

        # xh_all: normalized, free layout (m, c, f) with m = bb*2 + mi
        xh_all = sb.tile([NBLK, NMAT * MFREE + 8], f32)
        for bb in range(B_PER_CORE):
            for mi in range(2):
                off = moff(bb, mi)
                sfx = f"{bb}_{mi}"
                na2 = sb.tile([NBLK, BLK], f32, name=f"na2_{sfx}")
                sq1 = sb.tile([NBLK, BLK], f32, name=f"sq1_{sfx}")
                sq2 = sb.tile([NBLK, BLK], f32, name=f"sq2_{sfx}")
                xs = [xt_all[:, off + c * BLK: off + (c + 1) * BLK]
                      for c in range(C)]
                nc.vector.tensor_tensor(na2[:], xs[0], xs[0], OP.mult)
                nc.vector.tensor_tensor(sq1[:], xs[1], xs[1], OP.mult)
                nc.vector.tensor_tensor(sq2[:], xs[2], xs[2], OP.mult)
                nc.vector.tensor_tensor(na2[:], na2[:], sq1[:], OP.add)
                nc.vector.tensor_tensor(na2[:], na2[:], sq2[:], OP.add)

                # rn = 1/sqrt(na2): r = 1/na2 (iterative divide, accurate),
                # y0 = ACT sqrt(r), one Newton step y1 = 0.5*(y0 + r/y0)
                r = sb.tile([NBLK, BLK], f32, name=f"r_{sfx}")
                nc.vector.reciprocal(r[:], na2[:])
                y0 = sb.tile([NBLK, BLK], f32, name=f"y0_{sfx}")
                nc.scalar.sqrt(y0[:], r[:])
                iy = sb.tile([NBLK, BLK], f32, name=f"iy_{sfx}")
                nc.vector.reciprocal(iy[:], y0[:])
                nc.vector.tensor_tensor(iy[:], iy[:], r[:], OP.mult)
                nc.vector.tensor_tensor(iy[:], iy[:], y0[:], OP.add)
                rn = sb.tile([NBLK, BLK], f32, name=f"rn_{sfx}")
                nc.vector.tensor_scalar_mul(rn[:], iy[:], 0.5)

                for c in range(C):
                    nc.vector.tensor_tensor(
                        xh_all[:, off + c * BLK: off + (c + 1) * BLK],
                        xs[c], rn[:], OP.mult)

        # single-pass fp32 matmuls (4 cyc/row on PE, but exact):
        # one DRAM bounce rearranges the normalized clouds into one
        # [3, 4*4096] PE-operand tile at partitions 0-2
        scr = dr.tile([NMAT, NBLK, C, BLK], f32)
        nc.sync.dma_start(
            scr[:].rearrange("m j c f -> j m c f"),
            xh_all[:, 0:NMAT * MFREE]
            .rearrange("j (m c f) -> j m c f", c=C, f=BLK))
        x3_all = sb.tile([C, NMAT * N + 8], f32)
        nc.sync.dma_start(
            x3_all[:, 0:NMAT * N].rearrange("c (m j f) -> c m j f",
                                            j=NBLK, f=BLK),
            scr[:].rearrange("m j c f -> c m j f"))

        def x3(bb, mi):
            base = (mi * B_PER_CORE + bb) * N
            return x3_all[:, base:base + N]

        # ---- main: both directions per batch ----
        acc = sb.tile([128, 2 * B_PER_CORE], f32)
        k = 0
        for bb in range(B_PER_CORE):
            for (q, d) in ((0, 1), (1, 0)):
                qt = x3(bb, q)   # queries  [3, 4096]
                dt = x3(bb, d)   # database [3, 4096]
                rowparts = sb.tile([128, 2 * NI], f32, name=f"rp_{bb}_{q}")
                for i in range(NI):
                    lhsT = qt[:, i * 128:(i + 1) * 128]
                    for g in range(2):
                        psm = ps_pool.tile([128, 4 * BLK], f32, name="psm",
                                           tag="psm")
                        for jj in range(4):
                            blk = g * 4 + jj
                            rhs = dt[:, blk * BLK:(blk + 1) * BLK]
                            nc.tensor.matmul(
                                psm[:, jj * BLK:(jj + 1) * BLK],
                                lhsT=lhsT, rhs=rhs, start=True, stop=True)
                        nc.vector.reduce_max(
                            rowparts[:, 2 * i + g:2 * i + g + 1], psm[:],
                            axis=AX.X)
                # smax over the two groups, clamp, (1-s)^2, row-sum
                smax = sb.tile([128, NI], f32, name=f"sm_{bb}_{q}")
                nc.vector.reduce_max(
                    smax[:], rowparts[:].rearrange("p (i g) -> p i g", g=2),
                    axis=AX.X)
                nc.vector.tensor_scalar_min(smax[:], smax[:], 1.0)
                dd = sb.tile([128, NI], f32, name=f"dd_{bb}_{q}")
                nc.scalar.activation(dd[:], smax[:], AF.Square,
                                     bias=1.0, scale=-1.0)
                nc.vector.reduce_sum(acc[:, k:k + 1], dd[:], axis=AX.X)
                k += 1

        accf = sb.tile([128, 2], f32)
        nc.vector.reduce_sum(accf[:, 0:1], acc[:], axis=AX.X)
        nc.sync.dma_start(out.ap(), accf[:, 0:1])

    _split_fat_waits(nc)
    return nc


def _get_runner():
    """Build the Bass program and a REUSABLE jitted shard_map executable.

    run_bass_kernel_spmd (the axon path: bass2jax.run_bass_via_pjrt) builds a
    fresh jax.jit closure every call, so every kernel() invocation re-traces,
    re-lowers, and re-builds the PJRT executable (~250 ms).  The device
    program itself runs in well under a millisecond.  Here we do the same
    lowering ONCE and cache the compiled callable; repeat calls hit the jit
    C++ fast path (transfer + execute only).

    Per-core inputs are contiguous slices along axis 0, so the global
    concatenated arrays shard_map wants are exactly the full [B, C, N]
    host-transposed tensors — no per-call concat needed.
    """
    if "runner" in _CACHE:
        return _CACHE["runner"]

    import jax
    from jax.experimental.shard_map import shard_map
    from jax.sharding import Mesh, PartitionSpec
    from concourse import bass2jax
    import concourse.mybir as mybir

    nc = _build()
    bass2jax.install_neuronx_cc_hook()

    partition_name = (nc.partition_id_tensor.name
                      if nc.partition_id_tensor is not None else None)
    in_names, out_names, out_avals = [], [], []
    for alloc in nc.m.functions[0].allocations:
        if not isinstance(alloc, mybir.MemoryLocationSet):
            continue
        name = alloc.memorylocations[0].name
        if alloc.kind == "ExternalInput":
            if name != partition_name:
                in_names.append(name)
        elif alloc.kind == "ExternalOutput":
            shape = tuple(alloc.tensor_shape)
            dtype = mybir.dt.np(alloc.dtype)
            out_names.append(name)
            out_avals.append(jax.core.ShapedArray(shape, dtype))
    n_params = len(in_names)
    n_outs = len(out_names)
    all_in = in_names + out_names + ([partition_name] if partition_name else [])
    donate = tuple(range(n_params, n_params + n_outs))

    def _body(*args):
        operands = list(args)
        if partition_name is not None:
            operands.append(bass2jax.partition_id_tensor())
        outs = bass2jax._bass_exec_p.bind(
            *operands,
            out_avals=tuple(out_avals),
            in_names=tuple(all_in),
            out_names=tuple(out_names),
            lowering_input_output_aliases=(),
            sim_require_finite=True,
            sim_require_nnan=True,
            nc=nc,
        )
        return tuple(outs)

    devices = jax.devices()[:CORES]
    assert len(devices) == CORES, f"need {CORES} cores, saw {len(devices)}"
    mesh = Mesh(np.asarray(devices), ("core",))
    sharded = jax.jit(
        shard_map(_body, mesh=mesh,
                  in_specs=(PartitionSpec("core"),) * (n_params + n_outs),
                  out_specs=(PartitionSpec("core"),) * n_outs,
                  check_rep=False),
        donate_argnums=donate, keep_unused=True)
    zero_shapes = [((CORES * a.shape[0],) + tuple(a.shape[1:]), a.dtype)
                   for a in out_avals]
    _CACHE["runner"] = (sharded, in_names, zero_shapes)
    return _CACHE["runner"]


def kernel(matrix1: np.ndarray, matrix2: np.ndarray) -> np.ndarray:
    sharded, in_names, zero_shapes = _get_runner()

    m1t = np.ascontiguousarray(np.asarray(matrix1).transpose(0, 2, 1))  # [B,3,N]
    m2t = np.ascontiguousarray(np.asarray(matrix2).transpose(0, 2, 1))
    ins = {"m1t": m1t, "m2t": m2t}
    arrs = [ins[n] for n in in_names]
    zeros = [np.zeros(shape, dtype) for shape, dtype in zero_shapes]
    outs = sharded(*arrs, *zeros)
    total = np.asarray(outs[0]).sum(dtype=np.float64)
    return np.float32(total / (N * B))

